# revision 23
# baseline (speedup 1.0000x reference)
"""Sheaf Laplacian spectrum kernel for Trainium2 (8 NeuronCores, SPMD).

Strategy: row-shard L over 8 cores (64 nodes / 512 Laplacian rows per core).
Each core builds restriction maps R_e for the edges touching its nodes and
assembles its row-slab of L = delta^T @ delta as 64 (i,j) stalk-pair grids
via one-hot scatter matmuls on the TensorEngine (no inter-core collectives).
Host reassembles L (pure layout) and runs the symmetric eigensolver.
"""

from contextlib import ExitStack

import numpy as np

N, M, D_IN, S, T = 512, 4096, 256, 8, 8
NCORES = 8
NPC = N // NCORES  # nodes per core
SS = S * S
P = 128


def _build_program(EPAD, stage=99):
    """Emit the per-core Bass program. EPAD = padded length of each edge list.

    stage < 99 truncates the program for hardware bisection (debug only).
    """
    import concourse.bass as bass
    import concourse.mybir as mybir
    from concourse.bacc import Bacc
    from concourse.masks import make_identity
    from concourse.tile import TileContext

    AF = mybir.AluOpType
    ActF = mybir.ActivationFunctionType
    f32 = mybir.dt.float32
    EP2 = 2 * EPAD
    nTA = EPAD // P        # k-tiles per edge list
    nT = 2 * nTA           # total e-tiles (A list then B list)
    KD = D_IN // P         # k-tiles over the embedding dim (2)
    NN = N // P            # tiles over nodes (4)

    nc = Bacc()
    embT = nc.declare_dram_parameter("embT", [D_IN, N], f32, isOutput=False)
    wT = nc.declare_dram_parameter("wT", [D_IN, S], f32, isOutput=False)
    sdiff = nc.declare_dram_parameter("sdiff", [N, EP2], f32, isOutput=False)
    rwg = nc.declare_dram_parameter("rwg", [EP2, SS], f32, isOutput=False)
    gl = nc.declare_dram_parameter("gl", [EP2, NPC], f32, isOutput=False)
    gr1 = nc.declare_dram_parameter("gr1", [EP2, N], f32, isOutput=False)
    gr2 = nc.declare_dram_parameter("gr2", [EP2, N], f32, isOutput=False)
    grids = nc.declare_dram_parameter("grids", [SS, NPC, N], f32, isOutput=True)

    with ExitStack() as ctx:
        tc = ctx.enter_context(TileContext(nc))
        const = ctx.enter_context(tc.tile_pool(name="const", bufs=1))
        work = ctx.enter_context(tc.tile_pool(name="work", bufs=4))
        sgp = ctx.enter_context(tc.tile_pool(name="sgp", bufs=6))
        outp = ctx.enter_context(tc.tile_pool(name="outp", bufs=4))
        psg = ctx.enter_context(tc.tile_pool(name="psg", bufs=4, space="PSUM"))
        pss = ctx.enter_context(tc.tile_pool(name="pss", bufs=3, space="PSUM"))

        ident = const.tile([P, P], f32, tag="ident")
        make_identity(nc, ident)

        # ---- resident loads ----
        embT_sb = const.tile([P, KD, N], f32, tag="embT")
        nc.sync.dma_start(out=embT_sb, in_=embT[:].rearrange("(t p) n -> p t n", p=P))
        wT_sb = const.tile([P, KD, S], f32, tag="wT")
        nc.sync.dma_start(out=wT_sb, in_=wT[:].rearrange("(t p) s -> p t s", p=P))
        sdiff_sb = const.tile([P, NN, EP2], f32, tag="sdiff")
        nc.sync.dma_start(out=sdiff_sb, in_=sdiff[:].rearrange("(t p) e -> p t e", p=P))
        rwg_sb = const.tile([P, nT, SS], f32, tag="rwg")
        nc.sync.dma_start(out=rwg_sb, in_=rwg[:].rearrange("(t p) c -> p t c", p=P))
        gl_sb = const.tile([P, nT, NPC], f32, tag="gl")
        nc.sync.dma_start(out=gl_sb, in_=gl[:].rearrange("(t p) c -> p t c", p=P))
        gr1_sb = const.tile([P, nT, N], f32, tag="gr1")
        nc.sync.dma_start(out=gr1_sb, in_=gr1[:].rearrange("(t p) c -> p t c", p=P))
        gr2_sb = const.tile([P, nT, N], f32, tag="gr2")
        nc.sync.dma_start(out=gr2_sb, in_=gr2[:].rearrange("(t p) c -> p t c", p=P))

        # ---- X = node_embeddings @ proj_W.T, node-major [512, 8] ----
        X_sb = const.tile([P, NN, S], f32, tag="X")
        for ntile in range(NN):
            px = pss.tile([P, S], f32, tag="ps")
            for kt in range(KD):
                nc.tensor.matmul(
                    px,
                    lhsT=embT_sb[:, kt, ntile * P:(ntile + 1) * P],
                    rhs=wT_sb[:, kt, :],
                    start=(kt == 0),
                    stop=(kt == KD - 1),
                )
            nc.vector.tensor_copy(X_sb[:, ntile, :], px)
        if stage == 0:
            nc.sync.dma_start(out=grids[0][:, 0:S], in_=X_sb[0:NPC, 0, :])

        # ---- dT[s, e] = (X[dst] - X[src])^T via scatter matmul ----
        if stage >= 1:
            dT_sb = const.tile([S, EP2], f32, tag="dT")
            col = 0
            while col < EP2:
                cw = min(512, EP2 - col)
                pd = pss.tile([S, 512], f32, tag="ps")
                for kt in range(NN):
                    nc.tensor.matmul(
                        pd[:, :cw],
                        lhsT=X_sb[:, kt, :],
                        rhs=sdiff_sb[:, kt, col:col + cw],
                        start=(kt == 0),
                        stop=(kt == NN - 1),
                    )
                nc.vector.tensor_copy(dT_sb[:, col:col + cw], pd[:, :cw])
                col += cw
            if stage == 1:
                nc.sync.dma_start(out=grids[0][0:S, :], in_=dT_sb[:, 0:512])

        # ---- transpose to edge-major d [e, s] ----
        if stage >= 2:
            d_sb = const.tile([P, nT, S], f32, tag="d")
            for t in range(nT):
                pt = pss.tile([P, S], f32, tag="ps")
                nc.tensor.transpose(pt, dT_sb[:, t * P:(t + 1) * P], ident[:S, :S])
                nc.vector.tensor_copy(d_sb[:, t, :], pt)
            if stage == 2:
                nc.sync.dma_start(out=grids[0][:, 0:S], in_=d_sb[0:NPC, 0, :])

        # ---- per-edge restriction maps ----
        if stage >= 3:
            sub = stage - 30 if 30 <= stage < 40 else 9  # sub-stage within the chain
            dn_sb = const.tile([P, nT], f32, tag="dn")
            inv_sb = const.tile([P, nT], f32, tag="inv")
            alpha_sb = const.tile([P, nT], f32, tag="alpha")
            hhat_sb = const.tile([P, nT, S], f32, tag="hhat")
            dump_sb = hhat_sb
            if sub >= 3:
                nau_sb = const.tile([P, nT, S], f32, tag="nau")
            if sub >= 4:
                R_sb = const.tile([P, nT, SS], f32, tag="R")
                nR_sb = const.tile([P, nT, SS], f32, tag="nR")
                dump_sb = R_sb
            if sub >= 6:
                RtR_sb = const.tile([P, nT, SS], f32, tag="RtR")

            for t in range(nT):
                d_t = d_sb[:, t, :]
                junk = work.tile([P, S], f32, tag="junk")
                nsq = work.tile([P, 1], f32, tag="nsq")
                nc.vector.tensor_tensor(out=junk, in0=d_t, in1=d_t, op=AF.mult)
                nc.vector.reduce_sum(nsq, junk, axis=mybir.AxisListType.X)
                # d_norm = sqrt(nsq) + 1e-12 ; inv = 1/d_norm ; alpha = min(d_norm, 1)
                sq = work.tile([P, 1], f32, tag="sq")
                nc.scalar.activation(sq, nsq, ActF.Sqrt)
                nc.vector.tensor_scalar_add(dn_sb[:, t:t + 1], sq, 1e-12)
                nc.vector.reciprocal(inv_sb[:, t:t + 1], dn_sb[:, t:t + 1])
                nc.vector.tensor_scalar_min(alpha_sb[:, t:t + 1], dn_sb[:, t:t + 1], 1.0)
                nc.vector.tensor_scalar_mul(hhat_sb[:, t, :], d_t, inv_sb[:, t:t + 1])
                # u_i = sum_j Rw[i,j] * hhat[j] ; nau = -alpha * u
                if sub >= 2:
                    u_t = work.tile([P, S], f32, tag="u_t")
                    for i in range(S):
                        junk2 = work.tile([P, S], f32, tag="junk2")
                        nc.vector.tensor_tensor(
                            out=junk2, in0=rwg_sb[:, t, i * S:(i + 1) * S],
                            in1=hhat_sb[:, t, :], op=AF.mult)
                        nc.vector.reduce_sum(u_t[:, i:i + 1], junk2,
                                             axis=mybir.AxisListType.X)
                if sub >= 3:
                    nc.vector.tensor_scalar(
                        out=nau_sb[:, t, :], in0=u_t, scalar1=alpha_sb[:, t:t + 1],
                        scalar2=-1.0, op0=AF.mult, op1=AF.mult,
                    )
                # R = Rw - alpha * u hhat^T  (row i: Rw[i,:] + nau_i * hhat)
                if sub >= 4:
                    for i in range(S):
                        nc.vector.scalar_tensor_tensor(
                            out=R_sb[:, t, i * S:(i + 1) * S],
                            in0=hhat_sb[:, t, :],
                            scalar=nau_sb[:, t, i:i + 1],
                            in1=rwg_sb[:, t, i * S:(i + 1) * S],
                            op0=AF.mult, op1=AF.add,
                        )
                if sub >= 5:
                    nc.vector.tensor_scalar_mul(nR_sb[:, t, :], R_sb[:, t, :], -1.0)
                if sub >= 6:
                    # RtR[i,j] = sum_s R[s,i] R[s,j]
                    rtr3 = RtR_sb[:, t, :].rearrange("p (i j) -> p i j", i=S)
                    for s in range(S):
                        row = R_sb[:, t, s * S:(s + 1) * S]
                        bi = bass.AP(tensor=row.tensor, offset=row.offset,
                                     ap=[row.ap[0], [1, S], [0, S]])
                        bj = bass.AP(tensor=row.tensor, offset=row.offset,
                                     ap=[row.ap[0], [0, S], [1, S]])
                        if s == 0:
                            nc.vector.tensor_tensor(out=rtr3, in0=bi, in1=bj, op=AF.mult)
                        else:
                            sc3 = work.tile([P, S, S], f32, tag="sc3")
                            nc.vector.tensor_tensor(out=sc3, in0=bi, in1=bj, op=AF.mult)
                            nc.vector.tensor_tensor(out=rtr3, in0=rtr3, in1=sc3, op=AF.add)
            if stage == 3 or (30 <= stage < 40):
                nc.sync.dma_start(out=grids[0][:, 0:S], in_=dump_sb[0:NPC, 0, 0:S])

        # ---- Gram assembly: 64 (i,j) grids, 2 per PSUM tile ----
        if stage >= 4 and not (30 <= stage < 40):
            diag = [(i, i) for i in range(S)]
            offd = [(i, j) for i in range(S) for j in range(S) if i != j]
            pairs = [(diag[k], diag[k + 1]) for k in range(0, S, 2)]
            pairs += [(offd[k], offd[k + 1]) for k in range(0, len(offd), 2)]
            if stage == 4:
                pairs = pairs[:2]

            for (iL, jL), (iR, jR) in pairs:
                both_diag = (iL == jL)
                pg = psg.tile([P, N], f32, tag="pg")
                nmm = 3 * nTA + (nTA if both_diag else 0)
                mm = 0
                for kt in range(nT):
                    is_a = kt < nTA
                    # cross term: A rows scatter -R[iL jL] at (src, dst);
                    # B rows scatter -R^T = -R[j i] at (dst, src)
                    sg = sgp.tile([P, P], f32, tag="sg")
                    for half, (i, j) in ((0, (iL, jL)), (1, (iR, jR))):
                        fij = (i * S + j) if is_a else (j * S + i)
                        nc.vector.tensor_scalar_mul(
                            sg[:, half * NPC:(half + 1) * NPC],
                            gl_sb[:, kt, :], nR_sb[:, kt, fij:fij + 1])
                    nc.tensor.matmul(pg, lhsT=sg, rhs=gr1_sb[:, kt, :],
                                     start=(mm == 0), stop=(mm == nmm - 1))
                    mm += 1
                    if not is_a:
                        # (dst,dst) block: + R^T R
                        sg2 = sgp.tile([P, P], f32, tag="sg2")
                        for half, (i, j) in ((0, (iL, jL)), (1, (iR, jR))):
                            fij = i * S + j
                            nc.vector.tensor_scalar_mul(
                                sg2[:, half * NPC:(half + 1) * NPC],
                                gl_sb[:, kt, :], RtR_sb[:, kt, fij:fij + 1])
                        nc.tensor.matmul(pg, lhsT=sg2, rhs=gr2_sb[:, kt, :],
                                         start=(mm == 0), stop=(mm == nmm - 1))
                        mm += 1
                    elif both_diag:
                        # (src,src) block: + deg * I (identity stalk entries only)
                        sgd = sgp.tile([P, P], f32, tag="sgd")
                        nc.vector.tensor_copy(sgd[:, 0:NPC], gl_sb[:, kt, :])
                        nc.vector.tensor_copy(sgd[:, NPC:2 * NPC], gl_sb[:, kt, :])
                        nc.tensor.matmul(pg, lhsT=sgd, rhs=gr2_sb[:, kt, :],
                                         start=(mm == 0), stop=(mm == nmm - 1))
                        mm += 1
                ob = outp.tile([P, N], f32, tag="ob")
                nc.vector.tensor_copy(ob, pg)
                nc.sync.dma_start(out=grids[iL * S + jL], in_=ob[0:NPC, :])
                nc.sync.dma_start(out=grids[iR * S + jR], in_=ob[NPC:2 * NPC, :])

    nc.finalize()
    return nc


def _host_prep(node_embeddings, proj_W, R_weights, src, dst, etype):
    """Build per-core input maps. Index manipulation / layout only."""
    f32 = np.float32
    src = np.asarray(src).astype(np.int64)
    dst = np.asarray(dst).astype(np.int64)
    etype = np.asarray(etype).astype(np.int64)
    A_lists = [np.where((src // NPC == c) & (src != dst))[0] for c in range(NCORES)]
    B_lists = [np.where(dst // NPC == c)[0] for c in range(NCORES)]
    maxlen = max(max(len(a) for a in A_lists), max(len(b) for b in B_lists))
    EPAD = max(128, ((maxlen + P - 1) // P) * P)
    EP2 = 2 * EPAD

    embT = np.ascontiguousarray(np.asarray(node_embeddings).T.astype(f32))
    wT = np.ascontiguousarray(np.asarray(proj_W).T.astype(f32))
    Rw = np.asarray(R_weights).astype(f32)

    in_maps = []
    for c in range(NCORES):
        A, B = A_lists[c], B_lists[c]
        sdiff = np.zeros((N, EP2), f32)
        rwg = np.zeros((EP2, SS), f32)
        gl = np.zeros((EP2, NPC), f32)
        gr1 = np.zeros((EP2, N), f32)
        gr2 = np.zeros((EP2, N), f32)
        for base, lst in ((0, A), (EPAD, B)):
            k = np.arange(len(lst))
            s_, d_, t_ = src[lst], dst[lst], etype[lst]
            np.add.at(sdiff, (d_, base + k), 1.0)
            np.add.at(sdiff, (s_, base + k), -1.0)
            rwg[base + k] = Rw[t_].reshape(len(lst), SS)
            if base == 0:  # A list: rows = src local
                gl[k, s_ - c * NPC] = 1.0
                gr1[k, d_] = 1.0          # cross -R target
                gr2[k, s_] = 1.0          # deg target
            else:          # B list: rows = dst local
                gl[base + k, d_ - c * NPC] = 1.0
                nonself = s_ != d_
                gr1[base + k[nonself], s_[nonself]] = 1.0  # cross -R^T target
                gr2[base + k, d_] = 1.0   # RtR target
        in_maps.append({
            "embT": embT, "wT": wT,
            "sdiff": np.ascontiguousarray(sdiff),
            "rwg": rwg, "gl": gl, "gr1": gr1, "gr2": gr2,
        })
    return EPAD, in_maps


def _assemble_L(grids_per_core):
    slabs = []
    for g in grids_per_core:
        slab = g.reshape(S, S, NPC, N).transpose(2, 0, 3, 1).reshape(NPC * S, N * S)
        slabs.append(slab)
    return np.concatenate(slabs, axis=0)


LAST_RESULTS = None


def kernel(node_embeddings, proj_W, R_weights, src, dst, etype, _trace=False):
    global LAST_RESULTS
    from concourse.bass_utils import run_bass_kernel_spmd

    EPAD, in_maps = _host_prep(node_embeddings, proj_W, R_weights, src, dst, etype)
    nc = _build_program(EPAD)
    br = run_bass_kernel_spmd(nc, in_maps, list(range(NCORES)), trace=_trace)
    LAST_RESULTS = br
    res = br.results
    L = _assemble_L([np.asarray(res[c]["grids"]) for c in range(NCORES)])
    L = 0.5 * (L + L.T)
    try:
        import scipy.linalg as sla
        w = sla.eigh(L, eigvals_only=True, driver="evd")
    except Exception:
        w = np.linalg.eigvalsh(L)
    return np.maximum(w, 0.0).astype(np.float32)
